# revision 4
# baseline (speedup 1.0000x reference)
"""Trainium2 Bass kernel for CombineRadialSpeciesWithAngularAdaptBasis.

Computation: for l in 0..5 (m = 2l+1):
    o_l = einsum('smp,pb->smb', values_l [N,m,P], W_l [P,B])   -> reshape (N*m, B)
    g_l = einsum('sxmp,pb->sxmb', grads_l [NG,3,m,P], W_l)     -> reshape (NG*3*m, B)
  output = concat([o_0, g_0_flat? ...]) -- precisely concat([o_0, g_0, o_1, g_1, ... o_5, g_5], axis=0)
  (o_l flattened to (N*m, B), g_l to (NG*3*m, B))

Strategy: data-parallel across samples on 8 NeuronCores. Host transposes each
shard to X^T [P=80, S] layout; on-chip, W_l [80,64] is the stationary matmul
operand and X^T streams through the PE as the moving operand in 512-column
tiles, producing out^T [64, S] per core, which the host transposes back.
All per-l blocks are processed back-to-back inside one NEFF.
"""
import numpy as np

N, NG, P, B, LMAX = 30000, 8000, 80, 64, 5
NCORES = 8
NV = N // NCORES      # 3750 values samples per core
NGV = NG // NCORES    # 1000 grads samples per core

CHUNK = 4096          # columns per DMA chunk
NT = 512              # matmul moving-operand tile (one PSUM bank fp32)

# Region order matches the reference's output concatenation: v0,g0,v1,g1,...
# Each entry: (input name, l, columns per core)
REGIONS = []
for _l in range(LMAX + 1):
    _m = 2 * _l + 1
    REGIONS.append((f"vt{_l}", _l, NV * _m))
    REGIONS.append((f"gt{_l}", _l, NGV * 3 * _m))
STOT = sum(r[2] for r in REGIONS)  # 243000

_CACHE = {}


def _build_program():
    """Build and finalize the (SPMD, per-core) Bass program once."""
    import concourse.bass as bass
    import concourse.tile as tile
    import concourse.mybir as mybir
    from concourse import bacc

    f32 = mybir.dt.float32
    f32r = mybir.dt.float32r

    nc = bacc.Bacc("TRN2", target_bir_lowering=False, debug=False,
                   num_devices=NCORES)
    xins = {}
    for name, l, cols in REGIONS:
        xins[name] = nc.declare_dram_parameter(name, [P, cols], f32r,
                                               isOutput=False)
    wins = [nc.declare_dram_parameter(f"w{l}", [P, B], f32r, isOutput=False)
            for l in range(LMAX + 1)]
    y = nc.declare_dram_parameter("y", [B, STOT], f32, isOutput=True)

    with tile.TileContext(nc) as tc:
        with (
            tc.tile_pool(name="wp", bufs=1) as wp,
            tc.tile_pool(name="inp", bufs=6) as inp,
            tc.tile_pool(name="outp", bufs=3) as outp,
            tc.tile_pool(name="psp", bufs=8, space="PSUM") as psp,
        ):
            w_sb = []
            for l in range(LMAX + 1):
                wt = wp.tile([P, B], f32r, name=f"wt{l}", tag=f"wt{l}")
                nc.sync.dma_start(wt[:], wins[l][:, :])
                w_sb.append(wt)

            yoff = 0
            ci = 0   # chunk counter (names)
            pi = 0   # pair counter, for copy-engine alternation
            for name, l, cols in REGIONS:
                xd = xins[name]
                chunk_starts = list(range(0, cols, CHUNK))
                gi = 0
                while gi < len(chunk_starts):
                    # group up to 2 consecutive chunks into one output DMA
                    group = chunk_starts[gi:gi + 2]
                    gi += len(group)
                    g0 = group[0]
                    gsz = sum(min(CHUNK, cols - c0) for c0 in group)
                    ot = outp.tile([B, gsz], f32, name=f"ot_{ci}", tag="ot")
                    oo = 0
                    for c0 in group:
                        csz = min(CHUNK, cols - c0)
                        xt = inp.tile([P, csz], f32r, name=f"xt_{ci}",
                                      tag="xt")
                        nc.sync.dma_start(xt[:], xd[:, c0:c0 + csz])
                        for k0 in range(0, csz, NT):
                            n = min(NT, csz - k0)
                            ps = psp.tile([B, n], f32,
                                          name=f"ps_{ci}_{k0}", tag="ps")
                            nc.tensor.matmul(ps[:], lhsT=w_sb[l][:],
                                             rhs=xt[:, k0:k0 + n],
                                             start=True, stop=True)
                            # whole pair's copies on one engine so the output
                            # DMA carries a single sync wait
                            if pi % 2 == 0:
                                nc.vector.tensor_copy(ot[:, oo + k0:oo + k0 + n],
                                                      ps[:])
                            else:
                                nc.scalar.copy(ot[:, oo + k0:oo + k0 + n],
                                               ps[:])
                        oo += csz
                        ci += 1
                    nc.scalar.dma_start(y[:, yoff + g0:yoff + g0 + gsz],
                                        ot[:])
                    pi += 1
                yoff += cols

    nc.finalize()
    return nc


def _get_program():
    if "nc" not in _CACHE:
        _CACHE["nc"] = _build_program()
    return _CACHE["nc"]


def _register_ntff_hook():
    """antenv.axon_hooks is absent in this image; the .so supports NTFF
    profiling — install the shim so run_bass_kernel_spmd(trace=True) works."""
    import sys, types
    try:
        from antenv.axon_hooks import get_axon_ntff_profile_hook  # noqa: F401
        return
    except ImportError:
        pass
    import antenv
    from trn_agent_boot.trn_boot import _ntff_profile_via_ctypes
    mod = types.ModuleType("antenv.axon_hooks")
    mod._hook = _ntff_profile_via_ctypes('/opt/axon/libaxon_pjrt.so')
    mod.get_axon_ntff_profile_hook = lambda: mod._hook
    mod.set_axon_ntff_profile_hook = lambda h: setattr(mod, '_hook', h)
    sys.modules["antenv.axon_hooks"] = mod
    antenv.axon_hooks = mod


LAST_EXEC_TIME_NS = None
LAST_MEAN_EXEC_TIME_NS = None


def kernel(trace=False, trace_all_cores=False, **inputs):
    global LAST_EXEC_TIME_NS, LAST_MEAN_EXEC_TIME_NS
    from concourse.bass_utils import run_bass_kernel_spmd

    # ---- host-side shard + transpose to [P, S] per core ----
    in_maps = [dict() for _ in range(NCORES)]
    for l in range(LMAX + 1):
        m = 2 * l + 1
        v = np.asarray(inputs[f"values_l{l}"], dtype=np.float32)
        g = np.asarray(inputs[f"grads_l{l}"], dtype=np.float32)
        w = np.ascontiguousarray(np.asarray(inputs[f"W_l{l}"],
                                            dtype=np.float32))
        for i in range(NCORES):
            vs = v[i * NV:(i + 1) * NV].reshape(NV * m, P)
            gs = g[i * NGV:(i + 1) * NGV].reshape(NGV * 3 * m, P)
            in_maps[i][f"vt{l}"] = np.ascontiguousarray(vs.T)
            in_maps[i][f"gt{l}"] = np.ascontiguousarray(gs.T)
            in_maps[i][f"w{l}"] = w

    nc = _get_program()
    kwargs = {}
    if trace:
        _register_ntff_hook()
        kwargs["trace"] = True
        if trace_all_cores:
            kwargs["trace_cores"] = list(range(NCORES))
    res = run_bass_kernel_spmd(nc, in_maps, list(range(NCORES)), **kwargs)
    LAST_EXEC_TIME_NS = res.exec_time_ns
    LAST_MEAN_EXEC_TIME_NS = res.mean_exec_time_ns

    # ---- gather: transpose each region back and concatenate ----
    outs = [res.results[i]["y"] for i in range(NCORES)]
    total_rows = NCORES * STOT
    final = np.empty((total_rows, B), dtype=np.float32)
    row = 0
    off = 0
    for name, l, cols in REGIONS:
        for i in range(NCORES):
            final[row:row + cols] = outs[i][:, off:off + cols].T
            row += cols
        off += cols
    return final


# revision 5
# speedup vs baseline: 1.0623x; 1.0623x over previous
"""Trainium2 Bass kernel for CombineRadialSpeciesWithAngularAdaptBasis.

Computation: for l in 0..5 (m = 2l+1):
    o_l = einsum('smp,pb->smb', values_l [N,m,P], W_l [P,B])   -> reshape (N*m, B)
    g_l = einsum('sxmp,pb->sxmb', grads_l [NG,3,m,P], W_l)     -> reshape (NG*3*m, B)
  output = concat([o_0, g_0_flat? ...]) -- precisely concat([o_0, g_0, o_1, g_1, ... o_5, g_5], axis=0)
  (o_l flattened to (N*m, B), g_l to (NG*3*m, B))

Strategy: data-parallel across samples on 8 NeuronCores. Host transposes each
shard to X^T [P=80, S] layout; on-chip, W_l [80,64] is the stationary matmul
operand and X^T streams through the PE as the moving operand in 512-column
tiles, producing out^T [64, S] per core, which the host transposes back.
All per-l blocks are processed back-to-back inside one NEFF.
"""
import numpy as np

N, NG, P, B, LMAX = 30000, 8000, 80, 64, 5
NCORES = 8
NV = N // NCORES      # 3750 values samples per core
NGV = NG // NCORES    # 1000 grads samples per core

CHUNK = 4096          # columns per DMA chunk
NT = 512              # matmul moving-operand tile (one PSUM bank fp32)

# Region order matches the reference's output concatenation: v0,g0,v1,g1,...
# Each entry: (input name, l, columns per core)
REGIONS = []
for _l in range(LMAX + 1):
    _m = 2 * _l + 1
    REGIONS.append((f"vt{_l}", _l, NV * _m))
    REGIONS.append((f"gt{_l}", _l, NGV * 3 * _m))
STOT = sum(r[2] for r in REGIONS)  # 243000

_CACHE = {}


def _build_program():
    """Build and finalize the (SPMD, per-core) Bass program once."""
    import concourse.bass as bass
    import concourse.tile as tile
    import concourse.mybir as mybir
    from concourse import bacc

    f32 = mybir.dt.float32
    f32r = mybir.dt.float32r

    nc = bacc.Bacc("TRN2", target_bir_lowering=False, debug=False,
                   num_devices=NCORES)
    xins = {}
    for name, l, cols in REGIONS:
        xins[name] = nc.declare_dram_parameter(name, [P, cols], f32r,
                                               isOutput=False)
    wins = [nc.declare_dram_parameter(f"w{l}", [P, B], f32r, isOutput=False)
            for l in range(LMAX + 1)]
    y = nc.declare_dram_parameter("y", [B, STOT], f32, isOutput=True)

    with tile.TileContext(nc) as tc:
        with (
            tc.tile_pool(name="wp", bufs=1) as wp,
            tc.tile_pool(name="inp", bufs=6) as inp,
            tc.tile_pool(name="outp", bufs=5) as outp,
            tc.tile_pool(name="psp", bufs=8, space="PSUM") as psp,
        ):
            w_sb = []
            for l in range(LMAX + 1):
                wt = wp.tile([P, B], f32r, name=f"wt{l}", tag=f"wt{l}")
                nc.sync.dma_start(wt[:], wins[l][:, :])
                w_sb.append(wt)

            yoff = 0
            ci = 0  # chunk index, for copy-engine alternation
            for name, l, cols in REGIONS:
                xd = xins[name]
                for c0 in range(0, cols, CHUNK):
                    csz = min(CHUNK, cols - c0)
                    xt = inp.tile([P, csz], f32r, name=f"xt_{ci}", tag="xt")
                    nc.sync.dma_start(xt[:], xd[:, c0:c0 + csz])
                    ot = outp.tile([B, csz], f32, name=f"ot_{ci}", tag="ot")
                    for k0 in range(0, csz, NT):
                        n = min(NT, csz - k0)
                        ps = psp.tile([B, n], f32, name=f"ps_{ci}_{k0}",
                                      tag="ps")
                        nc.tensor.matmul(ps[:], lhsT=w_sb[l][:],
                                         rhs=xt[:, k0:k0 + n],
                                         start=True, stop=True)
                        # all copies of one chunk on one engine so the output
                        # DMA needs a single sync wait; alternate per chunk
                        if ci % 2 == 0:
                            nc.vector.tensor_copy(ot[:, k0:k0 + n], ps[:])
                        else:
                            nc.scalar.copy(ot[:, k0:k0 + n], ps[:])
                    nc.scalar.dma_start(y[:, yoff + c0:yoff + c0 + csz], ot[:])
                    ci += 1
                yoff += cols

    nc.finalize()
    return nc


def _get_program():
    if "nc" not in _CACHE:
        _CACHE["nc"] = _build_program()
    return _CACHE["nc"]


def _register_ntff_hook():
    """antenv.axon_hooks is absent in this image; the .so supports NTFF
    profiling — install the shim so run_bass_kernel_spmd(trace=True) works."""
    import sys, types
    try:
        from antenv.axon_hooks import get_axon_ntff_profile_hook  # noqa: F401
        return
    except ImportError:
        pass
    import antenv
    from trn_agent_boot.trn_boot import _ntff_profile_via_ctypes
    mod = types.ModuleType("antenv.axon_hooks")
    mod._hook = _ntff_profile_via_ctypes('/opt/axon/libaxon_pjrt.so')
    mod.get_axon_ntff_profile_hook = lambda: mod._hook
    mod.set_axon_ntff_profile_hook = lambda h: setattr(mod, '_hook', h)
    sys.modules["antenv.axon_hooks"] = mod
    antenv.axon_hooks = mod


LAST_EXEC_TIME_NS = None
LAST_MEAN_EXEC_TIME_NS = None


def kernel(trace=False, trace_all_cores=False, **inputs):
    global LAST_EXEC_TIME_NS, LAST_MEAN_EXEC_TIME_NS
    from concourse.bass_utils import run_bass_kernel_spmd

    # ---- host-side shard + transpose to [P, S] per core ----
    in_maps = [dict() for _ in range(NCORES)]
    for l in range(LMAX + 1):
        m = 2 * l + 1
        v = np.asarray(inputs[f"values_l{l}"], dtype=np.float32)
        g = np.asarray(inputs[f"grads_l{l}"], dtype=np.float32)
        w = np.ascontiguousarray(np.asarray(inputs[f"W_l{l}"],
                                            dtype=np.float32))
        for i in range(NCORES):
            vs = v[i * NV:(i + 1) * NV].reshape(NV * m, P)
            gs = g[i * NGV:(i + 1) * NGV].reshape(NGV * 3 * m, P)
            in_maps[i][f"vt{l}"] = np.ascontiguousarray(vs.T)
            in_maps[i][f"gt{l}"] = np.ascontiguousarray(gs.T)
            in_maps[i][f"w{l}"] = w

    nc = _get_program()
    kwargs = {}
    if trace:
        _register_ntff_hook()
        kwargs["trace"] = True
        if trace_all_cores:
            kwargs["trace_cores"] = list(range(NCORES))
    res = run_bass_kernel_spmd(nc, in_maps, list(range(NCORES)), **kwargs)
    LAST_EXEC_TIME_NS = res.exec_time_ns
    LAST_MEAN_EXEC_TIME_NS = res.mean_exec_time_ns

    # ---- gather: transpose each region back and concatenate ----
    outs = [res.results[i]["y"] for i in range(NCORES)]
    total_rows = NCORES * STOT
    final = np.empty((total_rows, B), dtype=np.float32)
    row = 0
    off = 0
    for name, l, cols in REGIONS:
        for i in range(NCORES):
            final[row:row + cols] = outs[i][:, off:off + cols].T
            row += cols
        off += cols
    return final
